# revision 8
# baseline (speedup 1.0000x reference)
"""Trainium2 Bass kernel for a single-layer multi-head self-attention.

Math per batch element b (one NeuronCore each):
    z[h] = W[h] @ x_b + b[h]          (d=32, L=1024) per head h in 0..7
    S    = z[h].T @ z[h] / sqrt(d)    (1024, 1024), symmetric since Q=K=V
    out[h] = softmax(S, axis=-1) @ z[h]   -> laid out (d, L) per head

Device layout:
  - z_all (256, 1024): heads stacked along partitions (d-major, group m holds
    heads 4m..4m+3 at partition rows 32s). Feeds score matmuls.
  - ztld  8 x (128, 256): token-major z (zt[tok, 32h+d] = z[h][d, tok]),
    computed by a second projection matmul (no on-device transposes);
    serves as the stationary operand of the AV matmuls.
  - Scores (K=32): 4 heads of a group are packed into the PE array via
    tile_position=(32s, 0) row groups -> 4 concurrent matmuls.
  - E = exp(S/sqrt(d)) unnormalized; symmetric E means row-chunks (l-chunk
    partitions, m free) serve directly as the AV rhs. Row sums come for free
    from the ACT accum_out during the exp.
  - AV (M=32 per head): 4 heads col-packed via tile_position=(0, 32s) into a
    single (128, 512) PSUM tile per column half -> output lands directly in
    the final (head-stacked) layout.
  - Normalization: reciprocal row-sums are partition-layout (128, 8) tiles;
    DMA roundtrip through DRAM transposes+broadcasts them to (32, 128) column
    slices; one tensor_tensor multiply per half.
"""

import math

import numpy as np

_B, _C, _P, _T = 8, 256, 16, 64
_H, _D = 8, 32
_L = _P * _T  # 1024
_NCORES = 8
_SCALE = 1.0 / math.sqrt(_D)

_CACHE = {}


def _build_nc():
    import concourse.bacc as bacc
    import concourse.mybir as mybir
    import concourse.tile as tile

    fp32 = mybir.dt.float32
    AF = mybir.ActivationFunctionType
    ALU = mybir.AluOpType

    nc = bacc.Bacc()

    x_d = nc.dram_tensor("x", [_C, _L], fp32, kind="ExternalInput")
    wt_d = nc.dram_tensor("wt", [_C, _C], fp32, kind="ExternalInput")
    btt_d = nc.dram_tensor("btt", [1, _C], fp32, kind="ExternalInput")
    bias_d = nc.dram_tensor("bias", [_C, 1], fp32, kind="ExternalInput")
    out_d = nc.dram_tensor("out", [_C, _L], fp32, kind="ExternalOutput")

    with tile.TileContext(nc) as tc:
        with (
            tc.tile_pool(name="consts", bufs=1) as consts,
            tc.tile_pool(name="xz", bufs=1) as xz,
            tc.tile_pool(name="epool", bufs=32) as epool,
            tc.tile_pool(name="accp", bufs=4) as accp,
            tc.tile_pool(name="small", bufs=2) as small,
            tc.tile_pool(name="pa", bufs=6, space="PSUM") as pa,
            tc.tile_pool(name="pb", bufs=2, space="PSUM") as pb,
            tc.tile_pool(name="dram", bufs=4, space="DRAM") as dram,
        ):
            # ---- load inputs ----
            x_sb, wt_sb, bias_sb = [], [], []
            for k in range(2):
                xk = xz.tile([128, _L], fp32, name=f"x{k}")
                nc.sync.dma_start(xk, x_d[128 * k : 128 * (k + 1), :])
                x_sb.append(xk)
                wtk = consts.tile([128, _C], fp32, name=f"wt{k}")
                nc.sync.dma_start(wtk, wt_d[128 * k : 128 * (k + 1), :])
                wt_sb.append(wtk)
                bk = consts.tile([128, 1], fp32, name=f"bias{k}")
                nc.sync.dma_start(bk, bias_d[128 * k : 128 * (k + 1), :])
                bias_sb.append(bk)
            btt_sb = consts.tile([1, _C], fp32, name="btt")
            nc.sync.dma_start(btt_sb, btt_d[:, :])
            ones_sb = consts.tile([1, 128], fp32, name="ones")
            nc.vector.memset(ones_sb, 1.0)

            # ---- z_all (256, 1024): heads stacked, projection + bias ----
            z_sb = []
            for m in range(2):
                zm = xz.tile([128, _L], fp32, name=f"z{m}")
                for half in range(2):
                    zp = pa.tile([128, 512], fp32, name=f"zp{m}_{half}", tag="pa")
                    for k in range(2):
                        nc.tensor.matmul(
                            zp,
                            wt_sb[k][:, 128 * m : 128 * (m + 1)],
                            x_sb[k][:, 512 * half : 512 * (half + 1)],
                            start=(k == 0),
                            stop=(k == 1),
                        )
                    nc.vector.tensor_scalar_add(
                        zm[:, 512 * half : 512 * (half + 1)], zp, bias_sb[m]
                    )
                z_sb.append(zm)

            # ---- ztld 8 x (128, 256): token-major z with bias ----
            zt_sb = []
            for j in range(8):
                ztp = pa.tile([128, 512], fp32, name=f"ztp{j}", tag="pa")
                for k in range(2):
                    nc.tensor.matmul(
                        ztp[:, 0:_C],
                        x_sb[k][:, 128 * j : 128 * (j + 1)],
                        wt_sb[k],
                        start=(k == 0),
                        stop=False,
                    )
                nc.tensor.matmul(
                    ztp[:, 0:_C], ones_sb, btt_sb, start=False, stop=True
                )
                ztj = xz.tile([128, _C], fp32, name=f"zt{j}")
                nc.vector.tensor_copy(ztj, ztp[:, 0:_C])
                zt_sb.append(ztj)

            # ---- attention, one group of 4 heads at a time ----
            for m in range(2):
                zg = z_sb[m]
                avh = [
                    pb.tile([128, 512], fp32, name=f"av{m}_{h}", tag="pb")
                    for h in range(2)
                ]
                racc = [
                    accp.tile([128, 16], fp32, name=f"racc{m}_{s}", tag="racc")
                    for s in range(4)
                ]
                es = {}

                def issue_av(j):
                    for half in range(2):
                        for s in range(4):
                            nc.tensor.matmul(
                                avh[half][32 * s : 32 * (s + 1), :],
                                zt_sb[j][:, 128 * m + 32 * s : 128 * m + 32 * (s + 1)],
                                es[(s, j)][:, 512 * half : 512 * (half + 1)],
                                start=(j == 0),
                                stop=(j == 7),
                                tile_position=(0, 32 * s),
                                skip_group_check=True,
                            )

                for i in range(8):
                    for half in range(2):
                        sps = []
                        for s in range(4):
                            sp = pa.tile(
                                [128, 512], fp32, name=f"sp{m}_{i}_{half}_{s}", tag="pa"
                            )
                            nc.tensor.matmul(
                                sp,
                                zg[32 * s : 32 * (s + 1), 128 * i : 128 * (i + 1)],
                                zg[32 * s : 32 * (s + 1), 512 * half : 512 * (half + 1)],
                                start=True,
                                stop=True,
                                tile_position=(32 * s, 0),
                            )
                            sps.append(sp)
                        for s in range(4):
                            if half == 0:
                                es[(s, i)] = epool.tile(
                                    [128, _L], fp32, name=f"e{m}_{s}_{i}", tag="e"
                                )
                            col = 8 * half + i
                            nc.scalar.activation(
                                es[(s, i)][:, 512 * half : 512 * (half + 1)],
                                sps[s],
                                AF.Exp,
                                scale=_SCALE,
                                accum_out=racc[s][:, col : col + 1],
                            )
                    if i >= 1:
                        issue_av(i - 1)
                issue_av(7)

                # ---- normalization ----
                rbt = small.tile([128, _L], fp32, name=f"rb{m}", tag="rb")
                for s in range(4):
                    rsum = small.tile([128, 8], fp32, name=f"rsum{m}_{s}", tag="rsum")
                    nc.vector.tensor_tensor(
                        rsum, racc[s][:, 0:8], racc[s][:, 8:16], op=ALU.add
                    )
                    rrec = small.tile([128, 8], fp32, name=f"rrec{m}_{s}", tag="rrec")
                    nc.vector.reciprocal(rrec, rsum)
                    dr = dram.tile([8, 128], fp32, name=f"dr{m}_{s}", tag="dr")
                    for i in range(8):
                        nc.sync.dma_start(
                            dr[i : i + 1, :].rearrange("a b -> b a"),
                            rrec[:, i : i + 1],
                        )
                    for i in range(8):
                        nc.sync.dma_start(
                            rbt[32 * s : 32 * (s + 1), 128 * i : 128 * (i + 1)],
                            dr[i : i + 1, :].to_broadcast([32, 128]),
                        )
                o = small.tile([128, _L], fp32, name=f"o{m}", tag="o")
                for half in range(2):
                    nc.vector.tensor_tensor(
                        o[:, 512 * half : 512 * (half + 1)],
                        avh[half],
                        rbt[:, 512 * half : 512 * (half + 1)],
                        op=ALU.mult,
                    )
                nc.sync.dma_start(out_d[128 * m : 128 * (m + 1), :], o)

    nc.finalize()
    return nc


def _get_compiled():
    if "nc" not in _CACHE:
        _CACHE["nc"] = _build_nc()
    return _CACHE["nc"]


def kernel(x: np.ndarray, W: np.ndarray, b: np.ndarray) -> np.ndarray:
    from concourse.bass_utils import run_bass_kernel_spmd

    x = np.ascontiguousarray(x, dtype=np.float32)
    W = np.ascontiguousarray(W, dtype=np.float32)
    b = np.ascontiguousarray(b, dtype=np.float32)

    wt = np.ascontiguousarray(W.reshape(_H * _D, _C).T)  # (C, H*D)
    btt = np.ascontiguousarray(b.reshape(1, _H * _D))
    bias = np.ascontiguousarray(b.reshape(_H * _D, 1))

    in_maps = [
        {
            "x": np.ascontiguousarray(x[i].reshape(_C, _L)),
            "wt": wt,
            "btt": btt,
            "bias": bias,
        }
        for i in range(_NCORES)
    ]

    nc = _get_compiled()
    res = run_bass_kernel_spmd(nc, in_maps, list(range(_NCORES)))
    out = np.stack(
        [res.results[i]["out"].reshape(_H * _D, _P, _T) for i in range(_NCORES)]
    )
    return out


# revision 10
# speedup vs baseline: 1.2627x; 1.2627x over previous
"""Trainium2 Bass kernel for a single-layer multi-head self-attention.

Math per batch element b (one NeuronCore each):
    z[h] = W[h] @ x_b + b[h]          (d=32, L=1024) per head h in 0..7
    S    = z[h].T @ z[h] / sqrt(d)    (1024, 1024), symmetric since Q=K=V
    out[h] = softmax(S, axis=-1) @ z[h]   -> laid out (d, L) per head

Device layout:
  - z_all (256, 1024): heads stacked along partitions (d-major, group m holds
    heads 4m..4m+3 at partition rows 32s). Feeds score matmuls.
  - ztld  8 x (128, 256): token-major z (zt[tok, 32h+d] = z[h][d, tok]),
    computed by a second projection matmul (no on-device transposes);
    serves as the stationary operand of the AV matmuls.
  - Scores (K=32): 4 heads of a group are packed into the PE array via
    tile_position=(32s, 0) row groups -> 4 concurrent matmuls.
  - E = exp(S/sqrt(d)) unnormalized; symmetric E means row-chunks (l-chunk
    partitions, m free) serve directly as the AV rhs. Row sums come for free
    from the ACT accum_out during the exp.
  - AV (M=32 per head): 4 heads col-packed via tile_position=(0, 32s) into a
    single (128, 512) PSUM tile per column half -> output lands directly in
    the final (head-stacked) layout.
  - Normalization: reciprocal row-sums are partition-layout (128, 8) tiles;
    DMA roundtrip through DRAM transposes+broadcasts them to (32, 128) column
    slices; one tensor_tensor multiply per half.
"""

import math

import numpy as np

_B, _C, _P, _T = 8, 256, 16, 64
_H, _D = 8, 32
_L = _P * _T  # 1024
_NCORES = 8
_SCALE = 1.0 / math.sqrt(_D)

_CACHE = {}


def _build_nc():
    import concourse.bacc as bacc
    import concourse.mybir as mybir
    import concourse.tile as tile

    fp32 = mybir.dt.float32
    AF = mybir.ActivationFunctionType
    ALU = mybir.AluOpType

    nc = bacc.Bacc()

    x_d = nc.dram_tensor("x", [_C, _L], fp32, kind="ExternalInput")
    wt_d = nc.dram_tensor("wt", [_C, _C], fp32, kind="ExternalInput")
    btt_d = nc.dram_tensor("btt", [1, _C], fp32, kind="ExternalInput")
    bias_d = nc.dram_tensor("bias", [_C, 1], fp32, kind="ExternalInput")
    out_d = nc.dram_tensor("out", [_C, _L], fp32, kind="ExternalOutput")

    with tile.TileContext(nc) as tc:
        with (
            tc.tile_pool(name="consts", bufs=1) as consts,
            tc.tile_pool(name="xz", bufs=1) as xz,
            tc.tile_pool(name="epool", bufs=32) as epool,
            tc.tile_pool(name="accp", bufs=4) as accp,
            tc.tile_pool(name="small", bufs=2) as small,
            tc.tile_pool(name="pa", bufs=6, space="PSUM") as pa,
            tc.tile_pool(name="pb", bufs=2, space="PSUM") as pb,
            tc.tile_pool(name="dram", bufs=4, space="DRAM") as dram,
        ):
            # ---- load inputs ----
            x_sb, wt_sb, bias_sb = [], [], []
            for k in range(2):
                xk = xz.tile([128, _L], fp32, name=f"x{k}")
                for half in range(2):
                    nc.sync.dma_start(
                        xk[:, 512 * half : 512 * (half + 1)],
                        x_d[128 * k : 128 * (k + 1), 512 * half : 512 * (half + 1)],
                    )
                x_sb.append(xk)
                wtk = consts.tile([128, _C], fp32, name=f"wt{k}")
                nc.sync.dma_start(wtk, wt_d[128 * k : 128 * (k + 1), :])
                wt_sb.append(wtk)
                bk = consts.tile([128, 1], fp32, name=f"bias{k}")
                nc.sync.dma_start(bk, bias_d[128 * k : 128 * (k + 1), :])
                bias_sb.append(bk)
            btt_sb = consts.tile([1, _C], fp32, name="btt")
            nc.sync.dma_start(btt_sb, btt_d[:, :])
            ones_sb = consts.tile([1, 128], fp32, name="ones")
            nc.vector.memset(ones_sb, 1.0)

            # ---- z_all (256, 1024): heads stacked, projection + bias ----
            z_sb = []
            for m in range(2):
                zm = xz.tile([128, _L], fp32, name=f"z{m}")
                for half in range(2):
                    zp = pa.tile([128, 512], fp32, name=f"zp{m}_{half}", tag="pa")
                    for k in range(2):
                        nc.tensor.matmul(
                            zp,
                            wt_sb[k][:, 128 * m : 128 * (m + 1)],
                            x_sb[k][:, 512 * half : 512 * (half + 1)],
                            start=(k == 0),
                            stop=(k == 1),
                        )
                    nc.vector.tensor_scalar_add(
                        zm[:, 512 * half : 512 * (half + 1)], zp, bias_sb[m]
                    )
                z_sb.append(zm)

            # ---- ztld 8 x (128, 256): token-major z with bias ----
            zt_sb = []
            for j in range(8):
                ztp = pa.tile([128, 512], fp32, name=f"ztp{j}", tag="pa")
                for k in range(2):
                    nc.tensor.matmul(
                        ztp[:, 0:_C],
                        x_sb[k][:, 128 * j : 128 * (j + 1)],
                        wt_sb[k],
                        start=(k == 0),
                        stop=False,
                    )
                nc.tensor.matmul(
                    ztp[:, 0:_C], ones_sb, btt_sb, start=False, stop=True
                )
                ztj = xz.tile([128, _C], fp32, name=f"zt{j}")
                nc.vector.tensor_copy(ztj, ztp[:, 0:_C])
                zt_sb.append(ztj)

            # ---- attention, one group of 4 heads at a time ----
            for m in range(2):
                zg = z_sb[m]
                avh = [
                    pb.tile([128, 512], fp32, name=f"av{m}_{h}", tag="pb")
                    for h in range(2)
                ]
                racc = [
                    accp.tile([128, 16], fp32, name=f"racc{m}_{s}", tag="racc")
                    for s in range(4)
                ]
                es = {}

                def issue_av(j):
                    for half in range(2):
                        for s in range(4):
                            nc.tensor.matmul(
                                avh[half][32 * s : 32 * (s + 1), :],
                                zt_sb[j][:, 128 * m + 32 * s : 128 * m + 32 * (s + 1)],
                                es[(s, j)][:, 512 * half : 512 * (half + 1)],
                                start=(j == 0),
                                stop=(j == 7),
                                tile_position=(0, 32 * s),
                                skip_group_check=True,
                            )

                for i in range(8):
                    for half in range(2):
                        sps = []
                        for s in range(4):
                            sp = pa.tile(
                                [128, 512], fp32, name=f"sp{m}_{i}_{half}_{s}", tag="pa"
                            )
                            nc.tensor.matmul(
                                sp,
                                zg[32 * s : 32 * (s + 1), 128 * i : 128 * (i + 1)],
                                zg[32 * s : 32 * (s + 1), 512 * half : 512 * (half + 1)],
                                start=True,
                                stop=True,
                                tile_position=(32 * s, 0),
                            )
                            sps.append(sp)
                        for s in range(4):
                            if half == 0:
                                es[(s, i)] = epool.tile(
                                    [128, _L], fp32, name=f"e{m}_{s}_{i}", tag="e"
                                )
                            col = 8 * half + i
                            nc.scalar.activation(
                                es[(s, i)][:, 512 * half : 512 * (half + 1)],
                                sps[s],
                                AF.Exp,
                                scale=_SCALE,
                                accum_out=racc[s][:, col : col + 1],
                            )
                    if i >= 1 and i <= 6:
                        issue_av(i - 1)

                # ---- rowsum reciprocal + broadcast, overlapped with last AVs
                rbt = small.tile([128, _L], fp32, name=f"rb{m}", tag="rb")
                for s in range(4):
                    rsum = small.tile([128, 8], fp32, name=f"rsum{m}_{s}", tag="rsum")
                    nc.vector.tensor_tensor(
                        rsum, racc[s][:, 0:8], racc[s][:, 8:16], op=ALU.add
                    )
                    rrec = small.tile([128, 8], fp32, name=f"rrec{m}_{s}", tag="rrec")
                    nc.vector.reciprocal(rrec, rsum)
                    dr = dram.tile([8, 128], fp32, name=f"dr{m}_{s}", tag="dr")
                    # transpose to l-order in DRAM: dr[i, p] = rrec[p, i]
                    nc.sync.dma_start(dr[:, :].rearrange("a b -> b a"), rrec)
                    # broadcast back: rbt[32s+d, 128i+p] = dr[i, p]
                    nc.sync.dma_start(
                        rbt[32 * s : 32 * (s + 1), :].rearrange(
                            "d (i q) -> d i q", i=8
                        ),
                        dr[:, :].unsqueeze(0).to_broadcast([32, 8, 128]),
                    )
                issue_av(6)
                issue_av(7)

                o = small.tile([128, _L], fp32, name=f"o{m}", tag="o")
                for half in range(2):
                    nc.vector.tensor_tensor(
                        o[:, 512 * half : 512 * (half + 1)],
                        avh[half],
                        rbt[:, 512 * half : 512 * (half + 1)],
                        op=ALU.mult,
                    )
                nc.sync.dma_start(out_d[128 * m : 128 * (m + 1), :], o)

    nc.finalize()
    return nc


def _get_compiled():
    if "nc" not in _CACHE:
        _CACHE["nc"] = _build_nc()
    return _CACHE["nc"]


def kernel(x: np.ndarray, W: np.ndarray, b: np.ndarray) -> np.ndarray:
    from concourse.bass_utils import run_bass_kernel_spmd

    x = np.ascontiguousarray(x, dtype=np.float32)
    W = np.ascontiguousarray(W, dtype=np.float32)
    b = np.ascontiguousarray(b, dtype=np.float32)

    wt = np.ascontiguousarray(W.reshape(_H * _D, _C).T)  # (C, H*D)
    btt = np.ascontiguousarray(b.reshape(1, _H * _D))
    bias = np.ascontiguousarray(b.reshape(_H * _D, 1))

    in_maps = [
        {
            "x": np.ascontiguousarray(x[i].reshape(_C, _L)),
            "wt": wt,
            "btt": btt,
            "bias": bias,
        }
        for i in range(_NCORES)
    ]

    nc = _get_compiled()
    res = run_bass_kernel_spmd(nc, in_maps, list(range(_NCORES)))
    out = np.stack(
        [res.results[i]["out"].reshape(_H * _D, _P, _T) for i in range(_NCORES)]
    )
    return out


# revision 11
# speedup vs baseline: 1.3184x; 1.0441x over previous
"""Trainium2 Bass kernel for a single-layer multi-head self-attention.

Math per batch element b (one NeuronCore each):
    z[h] = W[h] @ x_b + b[h]          (d=32, L=1024) per head h in 0..7
    S    = z[h].T @ z[h] / sqrt(d)    (1024, 1024), symmetric since Q=K=V
    out[h] = softmax(S, axis=-1) @ z[h]   -> laid out (d, L) per head

Device layout:
  - z_all (256, 1024): heads stacked along partitions (d-major, group m holds
    heads 4m..4m+3 at partition rows 32s). Feeds score matmuls.
  - ztld  8 x (128, 256): token-major z (zt[tok, 32h+d] = z[h][d, tok]),
    built from 16 PE transposes of z_all blocks (bias rides along).
  - Scores (K=32): 4 heads of a group are packed into the PE array via
    tile_position=(32s, 0) row groups -> 4 concurrent matmuls.
  - E = exp(S/sqrt(d)) unnormalized; symmetric E means row-chunks (l-chunk
    partitions, m free) serve directly as the AV rhs. Row sums are DVE
    tensor_reduce over full (128, 1024) E tiles (keeps ACT free of the
    per-instruction accumulator-readout cost).
  - AV (M=32 per head): 4 heads col-packed via tile_position=(0, 32s) into a
    single (128, 512) PSUM tile per column half -> output lands directly in
    the final (head-stacked) layout.
  - Normalization: reciprocal row-sums are partition-layout (128, 8) tiles;
    DMA roundtrip through DRAM transposes+broadcasts them to (32, 128) column
    slices; one tensor_tensor multiply per half.
"""

import math

import numpy as np

_B, _C, _P, _T = 8, 256, 16, 64
_H, _D = 8, 32
_L = _P * _T  # 1024
_NCORES = 8
_SCALE = 1.0 / math.sqrt(_D)

_CACHE = {}


def _build_nc():
    import concourse.bacc as bacc
    import concourse.mybir as mybir
    import concourse.tile as tile

    fp32 = mybir.dt.float32
    AF = mybir.ActivationFunctionType
    ALU = mybir.AluOpType
    AX = mybir.AxisListType

    nc = bacc.Bacc()

    x_d = nc.dram_tensor("x", [_C, _L], fp32, kind="ExternalInput")
    wt_d = nc.dram_tensor("wt", [_C, _C], fp32, kind="ExternalInput")
    bias_d = nc.dram_tensor("bias", [_C, 1], fp32, kind="ExternalInput")
    ident_d = nc.dram_tensor("ident", [128, 128], fp32, kind="ExternalInput")
    out_d = nc.dram_tensor("out", [_C, _L], fp32, kind="ExternalOutput")

    with tile.TileContext(nc) as tc:
        with (
            tc.tile_pool(name="consts", bufs=1) as consts,
            tc.tile_pool(name="xz", bufs=1) as xz,
            tc.tile_pool(name="epool", bufs=32) as epool,
            tc.tile_pool(name="accp", bufs=8) as accp,
            tc.tile_pool(name="small", bufs=2) as small,
            tc.tile_pool(name="pa", bufs=6, space="PSUM") as pa,
            tc.tile_pool(name="pb", bufs=2, space="PSUM") as pb,
            tc.tile_pool(name="dram", bufs=4, space="DRAM") as dram,
        ):
            # ---- load inputs (critical path first: x half0 + wt col0) ----
            xh = [
                [xz.tile([128, 512], fp32, name=f"x{k}_{h}") for h in range(2)]
                for k in range(2)
            ]
            wtc = [
                [consts.tile([128, 128], fp32, name=f"wt{k}_{m}") for m in range(2)]
                for k in range(2)
            ]
            for k in range(2):
                nc.sync.dma_start(xh[k][0], x_d[128 * k : 128 * (k + 1), 0:512])
            for k in range(2):
                nc.sync.dma_start(wtc[k][0], wt_d[128 * k : 128 * (k + 1), 0:128])
            bias_sb = []
            for k in range(2):
                bk = consts.tile([128, 1], fp32, name=f"bias{k}")
                nc.sync.dma_start(bk, bias_d[128 * k : 128 * (k + 1), :])
                bias_sb.append(bk)
            for k in range(2):
                nc.sync.dma_start(xh[k][1], x_d[128 * k : 128 * (k + 1), 512:1024])
            for k in range(2):
                nc.sync.dma_start(wtc[k][1], wt_d[128 * k : 128 * (k + 1), 128:256])
            ident_sb = consts.tile([128, 128], fp32, name="ident")
            nc.sync.dma_start(ident_sb, ident_d[:, :])

            # ---- z_all (256, 1024): heads stacked, projection + bias ----
            z_sb = []
            for m in range(2):
                zm = xz.tile([128, _L], fp32, name=f"z{m}")
                for half in range(2):
                    zp = pa.tile([128, 512], fp32, name=f"zp{m}_{half}", tag="pa")
                    for k in range(2):
                        nc.tensor.matmul(
                            zp,
                            wtc[k][m],
                            xh[k][half],
                            start=(k == 0),
                            stop=(k == 1),
                        )
                    nc.vector.tensor_scalar_add(
                        zm[:, 512 * half : 512 * (half + 1)], zp, bias_sb[m]
                    )
                z_sb.append(zm)

            # ---- ztld 8 x (128, 256): token-major z via PE transposes ----
            zt_sb = []
            for j in range(8):
                ztp = pa.tile([128, 512], fp32, name=f"ztp{j}", tag="pa")
                for k in range(2):
                    nc.tensor.transpose(
                        ztp[:, 128 * k : 128 * (k + 1)],
                        z_sb[k][:, 128 * j : 128 * (j + 1)],
                        ident_sb,
                    )
                ztj = xz.tile([128, _C], fp32, name=f"zt{j}")
                nc.vector.tensor_copy(ztj, ztp[:, 0:_C])
                zt_sb.append(ztj)

            # ---- attention, one group of 4 heads at a time ----
            for m in range(2):
                zg = z_sb[m]
                avh = [
                    pb.tile([128, 512], fp32, name=f"av{m}_{h}", tag="pb")
                    for h in range(2)
                ]
                rs8 = [
                    accp.tile([128, 8], fp32, name=f"rs8{m}_{s}", tag="rs8")
                    for s in range(4)
                ]
                es = {}

                def issue_av(j, halves=(0, 1)):
                    for half in halves:
                        for s in range(4):
                            nc.tensor.matmul(
                                avh[half][32 * s : 32 * (s + 1), :],
                                zt_sb[j][:, 128 * m + 32 * s : 128 * m + 32 * (s + 1)],
                                es[(s, j)][:, 512 * half : 512 * (half + 1)],
                                start=(j == 0),
                                stop=(j == 7),
                                tile_position=(0, 32 * s),
                                skip_group_check=True,
                            )

                for i in range(8):
                    for half in range(2):
                        sps = []
                        for s in range(4):
                            sp = pa.tile(
                                [128, 512], fp32, name=f"sp{m}_{i}_{half}_{s}", tag="pa"
                            )
                            nc.tensor.matmul(
                                sp,
                                zg[32 * s : 32 * (s + 1), 128 * i : 128 * (i + 1)],
                                zg[32 * s : 32 * (s + 1), 512 * half : 512 * (half + 1)],
                                start=True,
                                stop=True,
                                tile_position=(32 * s, 0),
                            )
                            sps.append(sp)
                        for s in range(4):
                            if half == 0:
                                es[(s, i)] = epool.tile(
                                    [128, _L], fp32, name=f"e{m}_{s}_{i}", tag="e"
                                )
                            nc.scalar.activation(
                                es[(s, i)][:, 512 * half : 512 * (half + 1)],
                                sps[s],
                                AF.Exp,
                                scale=_SCALE,
                            )
                    for s in range(4):
                        nc.vector.tensor_reduce(
                            rs8[s][:, i : i + 1], es[(s, i)], axis=AX.X, op=ALU.add
                        )
                    if i >= 1 and i <= 6:
                        issue_av(i - 1)

                # ---- rowsum reciprocal + broadcast, overlapped with last AVs
                rbt = small.tile([128, _L], fp32, name=f"rb{m}", tag="rb")
                for s in range(4):
                    rrec = small.tile([128, 8], fp32, name=f"rrec{m}_{s}", tag="rrec")
                    nc.vector.reciprocal(rrec, rs8[s])
                    dr = dram.tile([8, 128], fp32, name=f"dr{m}_{s}", tag="dr")
                    # transpose to l-order in DRAM: dr[i, p] = rrec[p, i]
                    nc.sync.dma_start(dr[:, :].rearrange("a b -> b a"), rrec)
                    # broadcast back: rbt[32s+d, 128i+p] = dr[i, p]
                    nc.sync.dma_start(
                        rbt[32 * s : 32 * (s + 1), :].rearrange(
                            "d (i q) -> d i q", i=8
                        ),
                        dr[:, :].unsqueeze(0).to_broadcast([32, 8, 128]),
                    )
                issue_av(6)

                o = small.tile([128, _L], fp32, name=f"o{m}", tag="o")
                for half in range(2):
                    issue_av(7, halves=(half,))
                    nc.vector.tensor_tensor(
                        o[:, 512 * half : 512 * (half + 1)],
                        avh[half],
                        rbt[:, 512 * half : 512 * (half + 1)],
                        op=ALU.mult,
                    )
                    nc.sync.dma_start(
                        out_d[
                            128 * m : 128 * (m + 1), 512 * half : 512 * (half + 1)
                        ],
                        o[:, 512 * half : 512 * (half + 1)],
                    )

    nc.finalize()
    return nc


def _get_compiled():
    if "nc" not in _CACHE:
        _CACHE["nc"] = _build_nc()
    return _CACHE["nc"]


def kernel(x: np.ndarray, W: np.ndarray, b: np.ndarray) -> np.ndarray:
    from concourse.bass_utils import run_bass_kernel_spmd

    x = np.ascontiguousarray(x, dtype=np.float32)
    W = np.ascontiguousarray(W, dtype=np.float32)
    b = np.ascontiguousarray(b, dtype=np.float32)

    wt = np.ascontiguousarray(W.reshape(_H * _D, _C).T)  # (C, H*D)
    bias = np.ascontiguousarray(b.reshape(_H * _D, 1))
    ident = np.eye(128, dtype=np.float32)

    in_maps = [
        {
            "x": np.ascontiguousarray(x[i].reshape(_C, _L)),
            "wt": wt,
            "bias": bias,
            "ident": ident,
        }
        for i in range(_NCORES)
    ]

    nc = _get_compiled()
    res = run_bass_kernel_spmd(nc, in_maps, list(range(_NCORES)))
    out = np.stack(
        [res.results[i]["out"].reshape(_H * _D, _P, _T) for i in range(_NCORES)]
    )
    return out
